# revision 44
# baseline (speedup 1.0000x reference)
"""Trainium2 Bass kernel for nn_RSA_layer (RSA relational self-attention layer).

The reference builds a [W, W, U] sim/softmax tensor but returns only row
i = W-1 of the weighted sum c. Two algebraic reductions make the kernel
tiny:

1. Only query row i = W-1 matters, and the softmax terms constant over the
   key axis j (proj_hj[i, u] and b[u]) cancel in the softmax ratio, so
       s[j, u] = (fs @ w_hi)[j, u] + (fs[W-1] . fs[j]) * w_dot[u]
2. The rank-1 dot-product term folds into the matmul weights:
       s = ((w_hi + outer(q, w_dot)).T @ NS)           with q = fs[W-1]
   where NS = new_state = [state[:, 1:] | input^T]  ([U, W], u on
   partitions, j on the free axis).

Then per unit u (one SBUF partition), softmax over j is a row softmax:
   c[u] = sum_j NS[u,j] e^{s[u,j]} / sum_j e^{s[u,j]}
computed without max subtraction (|s| <= ~30, safely inside f32 range).

Implementation notes (HW-profile driven):
- All DMAs are contiguous HWDGE transfers. Partition-scatter/gather DMAs
  ([1,128] <-> [128,1]) cost ~6-7 us (128 x 4B descriptors) and are
  replaced by tiny PE matmuls: q column via matmul with a [1,1] ones
  tile, output row via matmul with a [128,128] identity.
- The score matmul runs in float32r (1 cycle/row vs fp32's 4). f32r
  operands must be "rounded": the Pool engine copy-casts the f32 NS
  tiles to f32r off the critical path.
- exp runs on ACT reading PSUM directly with accum_out producing the
  softmax denominator for free; the numerator is a fused
  scalar_tensor_tensor (out=(NS*1.0)*E, accum_out=sum) on DVE.

SPMD strategy: the problem is ~650 KB of traffic and ~35 MFLOP - far
below the ~7-20 us on-chip collective latency floor - so each of the 8
cores computes the full (reduced) answer independently and core 0's
output is returned. No cross-core communication.
"""

import re

import numpy as np

W = 1024
U = 128
N_CORES = 8
CHUNK = 512
N_CHUNK = W // CHUNK

_cache: dict = {}


def _patched_tile_context():
    import concourse.tile as tile

    class PatchedTileContext(tile.TileContext):
        """TileContext whose tail drain splits its semaphore waits.

        The walrus build here encodes at most 1 sem wait per regular
        instruction (2 per EventSemaphore), but stock Tile attaches every
        outstanding sem wait to one tail Drain. Emit standalone 2-wait
        EventSemaphore SP instructions covering the final vector clock
        instead, then a bare drain. Also stashes live sem handles for the
        post-exit multi-wait splitting pass (_split_multiwaits).
        """

        def _drain_and_barrier(self, tick_clock, wait_clock):
            nc = self.nc
            self.sem_handles = {h.name: h for h in self.sems.allocated().values()}
            ticks = [int(v) for v in re.findall(r"\d+", repr(tick_clock.global_clock))]
            allocated = self.sems.allocated()
            waits = []
            for proc, handle in allocated.items():
                tick = ticks[proc] if proc < len(ticks) else 0
                if tick <= 0:
                    continue
                mult = 16 if handle.name.startswith("DMA") else 1
                waits.append((handle, tick * mult))
            for i in range(0, len(waits), 2):
                pair = waits[i : i + 2]
                ins = nc.sync.wait_ge(pair[0][0], pair[0][1])
                if len(pair) > 1:
                    ins._wait_ge(pair[1][0], pair[1][1])
            nc.sync.drain()
            # SP has now observed every engine's and DMA queue's final sem
            # value, so all work is done. A single SP->Pool handoff orders
            # the semaphore clear; the two stock all-engine barriers
            # (~2-3 us) are redundant.
            handoff = nc.alloc_semaphore("tail_handoff")
            nc.sync.sem_inc(handoff, 1)
            nc.gpsimd.wait_ge(handoff, 1)
            popped = nc._tile_sem_poison_stack.pop()
            assert popped is self._sem_poison
            nc.clear_and_free_semaphores(list(allocated.values()) + [handoff])

    return PatchedTileContext


def _split_multiwaits(nc, tc):
    """Move excess sem waits (>1 per instruction) onto same-engine
    EventSemaphore carriers inserted immediately before the consumer."""
    from concourse import mybir

    handles = tc.sem_handles
    eng_map = {
        mybir.EngineType.PE: nc.tensor,
        mybir.EngineType.DVE: nc.vector,
        mybir.EngineType.Activation: nc.scalar,
        mybir.EngineType.Pool: nc.gpsimd,
        mybir.EngineType.SP: nc.sync,
    }
    for f in nc.m.functions:
        for b in f.blocks:
            newlist = []
            changed = False
            for ins in list(b.instructions):
                si = ins.sync_info
                waits = list(si.on_wait) if si is not None and si.on_wait else []
                # EventSemaphore legally holds 2 waits (our own tail
                # carriers); don't touch it or rebuild its block.
                if type(ins).__name__ == "InstEventSemaphore":
                    newlist.append(ins)
                    continue
                if len(waits) > 1 and ins.engine in eng_map:
                    changed = True
                    extra, keep = waits[:-1], waits[-1:]
                    eng = eng_map[ins.engine]
                    for i in range(0, len(extra), 2):
                        pair = extra[i : i + 2]
                        carrier = eng.wait_ge(
                            handles[pair[0].ant_name], pair[0].wait_value
                        )
                        if len(pair) > 1:
                            carrier._wait_ge(
                                handles[pair[1].ant_name], pair[1].wait_value
                            )
                        # wait_ge appended the carrier to the current bb;
                        # pop it off there and splice it in before `ins`.
                        cb = nc.cur_bb.bb
                        cl = list(cb.instructions)
                        assert cl[-1].name == carrier.ins.name
                        cb.instructions = cl[:-1]
                        newlist.append(carrier.ins)
                    ins.sync_info = mybir.SyncInfo(on_wait=keep, on_update=si.on_update)
                newlist.append(ins)
            if changed:
                b.instructions = newlist


def _build():
    import concourse.bass as bass
    from concourse import mybir
    from concourse.masks import make_identity

    f32 = mybir.dt.float32
    f32r = mybir.dt.float32r

    nc = bass.Bass("TRN2", target_bir_lowering=False, debug=False, num_devices=N_CORES)
    inp = nc.dram_tensor("input_tensor", [1, U], f32, kind="ExternalInput").ap()
    state = nc.dram_tensor("state", [U, W], f32, kind="ExternalInput").ap()
    w = nc.dram_tensor("w", [2 * U + 1, U], f32, kind="ExternalInput").ap()
    out = nc.dram_tensor("out", [1, U], f32, kind="ExternalOutput").ap()

    PatchedTileContext = _patched_tile_context()
    with PatchedTileContext(nc) as tc:
        with (
            tc.tile_pool(name="data", bufs=1) as data,
            tc.tile_pool(name="work", bufs=2) as work,
            tc.tile_pool(name="psum", bufs=1, space="PSUM") as psum_pool,
        ):
            # --- loads: every DMA is a plain contiguous transfer. One
            # dma_start costs ~650ns of descriptor generation on its issuing
            # sequencer, so spread the gens across idle engine sequencers to
            # run them in parallel instead of serially on Sync.
            # One HWDGE/SWDGE stream moves only ~80-90 GB/s, so split the
            # 512KB state across four concurrent streams. Small dependency
            # DMAs are issued FIRST on their queue (queue FIFO order would
            # otherwise park them behind the big state transfers).
            # sync: inrow then state chunk 3; scalar: w_hi then wdot;
            # gpsimd/SWDGE (round-robin queues): state chunks 0-2.
            # State split into 6 concurrent ~85KB column streams, 2 per
            # issuing engine, interleaved so each compute half is fed by
            # each engine's FIRST state transfer. (Partition-split streams
            # with fat 4KB descriptors measured WORSE - a <128-partition DMA
            # only drives a fraction of the SBUF write ports.) Last NS
            # column (= input row transposed) comes from the PE q-column.
            # Issue order per engine: first-half state stream, then the
            # small dependency DMA, then the second-half stream - the first
            # stream's descriptor gen is the critical one, and the small
            # loads still complete well before their consumers need them.
            ns = data.tile([U, W], f32, tag="ns")
            inrow = data.tile([1, U], f32, tag="inrow")
            wdot = data.tile([1, U], f32, tag="wdot")
            w_hi = data.tile([U, U], f32, tag="w_hi")
            six = [0, 171, 342, 512, 683, 853, 1023]

            def stream(eng, k):
                lo, hi = six[k], six[k + 1]
                eng.dma_start(out=ns[:, lo:hi], in_=state[:, lo + 1 : hi + 1])

            stream(nc.gpsimd, 0)
            stream(nc.sync, 1)
            stream(nc.scalar, 2)
            nc.gpsimd.dma_start(out=w_hi[:], in_=w[0:U, :])
            nc.sync.dma_start(out=inrow[:], in_=inp[:], single_packet=True)
            nc.scalar.dma_start(
                out=wdot[:], in_=w[2 * U : 2 * U + 1, :], single_packet=True
            )
            stream(nc.gpsimd, 3)
            stream(nc.sync, 4)
            stream(nc.scalar, 5)

            # constants on otherwise-idle engines
            one = data.tile([1, 1], f32, tag="one")
            nc.vector.memset(one[:], 1.0)
            ident = data.tile([U, U], f32, tag="ident")
            make_identity(nc, ident[:])

            # Preload the ACT exp table at kernel start (otherwise the
            # ~1.5us table load serializes in front of the first real exp).
            act_warm = data.tile([1, 1], f32, tag="act_warm")
            nc.scalar.activation(
                act_warm[:], one[:], mybir.ActivationFunctionType.Exp
            )

            # q column [U,1] = input row transposed, via K=1 matmul with 1.0
            q_psum = psum_pool.tile([U, 1], f32, tag="q_psum")
            nc.tensor.matmul(q_psum[:], lhsT=inrow[:], rhs=one[:], start=True, stop=True)
            nc.vector.tensor_copy(ns[:, W - 1 : W], q_psum[:])

            # M_eff = w_hi + outer(q, w_dot); outer via K=1 matmul on PE
            outer_psum = psum_pool.tile([U, U], f32, tag="outer")
            nc.tensor.matmul(
                outer_psum[:], lhsT=inrow[:], rhs=wdot[:], start=True, stop=True
            )
            meff = data.tile([U, U], f32r, tag="meff")
            nc.vector.tensor_add(meff[:], w_hi[:], outer_psum[:])

            # Keep the PE clock ramped while waiting for the state DMA: a few
            # dummy K=1 matmuls into a scratch bank (PE would otherwise idle
            # ~1us and drop back to the slow p-state for the big matmuls).
            warm_psum = psum_pool.tile([U, U], f32, tag="warm")
            for _ in range(2):
                nc.tensor.matmul(
                    warm_psum[:], lhsT=inrow[:], rhs=wdot[:], start=True, stop=True
                )

            # f32r copy of NS for the PE (f32r needs pre-rounded input), on
            # DVE (Pool's CAST path measured ~3.7 ns/elem; DVE is ~1 ns/elem).
            nsr = data.tile([U, W], f32r, tag="nsr")
            warmed = False

            l_all = data.tile([U, N_CHUNK], f32, tag="l_all")
            num_all = data.tile([U, N_CHUNK], f32, tag="num_all")

            for c in range(N_CHUNK):
                lo, hi = c * CHUNK, (c + 1) * CHUNK
                nc.vector.tensor_copy(nsr[:, lo:hi], ns[:, lo:hi])
                ps = psum_pool.tile([U, CHUNK], f32, tag=f"ps{c}")
                nc.tensor.matmul(
                    ps[:], lhsT=meff[:], rhs=nsr[:, lo:hi], start=True, stop=True
                )
                e = work.tile([U, CHUNK], f32, tag="e")
                nc.scalar.activation(
                    e[:],
                    ps[:],
                    mybir.ActivationFunctionType.Exp,
                    accum_out=l_all[:, c : c + 1],
                )
                # num_c[u] = sum_j NS[u,j]*E[u,j]: out=(NS*1.0)*E, accum=sum
                t = work.tile([U, CHUNK], f32, tag="t")
                nc.vector.scalar_tensor_tensor(
                    out=t[:],
                    in0=ns[:, lo:hi],
                    scalar=1.0,
                    in1=e[:],
                    op0=mybir.AluOpType.mult,
                    op1=mybir.AluOpType.mult,
                    accum_out=num_all[:, c : c + 1],
                )

            l_sum = data.tile([U, 1], f32, tag="l_sum")
            nc.vector.reduce_sum(l_sum[:], l_all[:], axis=mybir.AxisListType.X)
            num_sum = data.tile([U, 1], f32, tag="num_sum")
            nc.vector.reduce_sum(num_sum[:], num_all[:], axis=mybir.AxisListType.X)
            r = data.tile([U, 1], f32, tag="r")
            nc.vector.reciprocal(r[:], l_sum[:])
            cfin = data.tile([U, 1], f32, tag="cfin")
            nc.vector.tensor_mul(cfin[:], num_sum[:], r[:])

            # transpose c to a contiguous [1,U] row on the PE, then one
            # contiguous 512B DMA out, straight from PSUM
            crow_psum = psum_pool.tile([1, U], f32, tag="crow")
            nc.tensor.matmul(
                crow_psum[:], lhsT=cfin[:], rhs=ident[:], start=True, stop=True
            )
            crow = data.tile([1, U], f32, tag="crow_sb")
            nc.scalar.copy(crow[:], crow_psum[:])
            nc.sync.dma_start(out=out[:], in_=crow[:], single_packet=True)

    _split_multiwaits(nc, tc)
    return nc


def _get_nc():
    if "nc" not in _cache:
        _cache["nc"] = _build()
    return _cache["nc"]


def kernel(**inputs) -> np.ndarray:
    from concourse.bass_utils import run_bass_kernel_spmd

    nc = _get_nc()
    in_map = {
        "input_tensor": np.ascontiguousarray(inputs["input_tensor"], dtype=np.float32),
        "state": np.ascontiguousarray(inputs["state"], dtype=np.float32),
        "w": np.ascontiguousarray(inputs["w"], dtype=np.float32),
    }
    in_maps = [in_map for _ in range(N_CORES)]
    res = run_bass_kernel_spmd(nc, in_maps, list(range(N_CORES)))
    return np.asarray(res.results[0]["out"], dtype=np.float32)


# revision 45
# speedup vs baseline: 1.1051x; 1.1051x over previous
"""Trainium2 Bass kernel for nn_RSA_layer (RSA relational self-attention layer).

The reference builds a [W, W, U] sim/softmax tensor but returns only row
i = W-1 of the weighted sum c. Two algebraic reductions make the kernel
tiny:

1. Only query row i = W-1 matters, and the softmax terms constant over the
   key axis j (proj_hj[i, u] and b[u]) cancel in the softmax ratio, so
       s[j, u] = (fs @ w_hi)[j, u] + (fs[W-1] . fs[j]) * w_dot[u]
2. The rank-1 dot-product term folds into the matmul weights:
       s = ((w_hi + outer(q, w_dot)).T @ NS)           with q = fs[W-1]
   where NS = new_state = [state[:, 1:] | input^T]  ([U, W], u on
   partitions, j on the free axis).

Then per unit u (one SBUF partition), softmax over j is a row softmax:
   c[u] = sum_j NS[u,j] e^{s[u,j]} / sum_j e^{s[u,j]}
computed without max subtraction (|s| <= ~30, safely inside f32 range).

Implementation notes (HW-profile driven):
- All DMAs are contiguous HWDGE transfers. Partition-scatter/gather DMAs
  ([1,128] <-> [128,1]) cost ~6-7 us (128 x 4B descriptors) and are
  replaced by tiny PE matmuls: q column via matmul with a [1,1] ones
  tile, output row via matmul with a [128,128] identity.
- The score matmul runs in float32r (1 cycle/row vs fp32's 4). f32r
  operands must be "rounded": the Pool engine copy-casts the f32 NS
  tiles to f32r off the critical path.
- exp runs on ACT reading PSUM directly with accum_out producing the
  softmax denominator for free; the numerator is a fused
  scalar_tensor_tensor (out=(NS*1.0)*E, accum_out=sum) on DVE.

SPMD strategy: the problem is ~650 KB of traffic and ~35 MFLOP - far
below the ~7-20 us on-chip collective latency floor - so each of the 8
cores computes the full (reduced) answer independently and core 0's
output is returned. No cross-core communication.
"""

import re

import numpy as np

W = 1024
U = 128
N_CORES = 8
CHUNK = 512
N_CHUNK = W // CHUNK

_cache: dict = {}


def _patched_tile_context():
    import concourse.tile as tile

    class PatchedTileContext(tile.TileContext):
        """TileContext whose tail drain splits its semaphore waits.

        The walrus build here encodes at most 1 sem wait per regular
        instruction (2 per EventSemaphore), but stock Tile attaches every
        outstanding sem wait to one tail Drain. Emit standalone 2-wait
        EventSemaphore SP instructions covering the final vector clock
        instead, then a bare drain. Also stashes live sem handles for the
        post-exit multi-wait splitting pass (_split_multiwaits).
        """

        def _drain_and_barrier(self, tick_clock, wait_clock):
            nc = self.nc
            self.sem_handles = {h.name: h for h in self.sems.allocated().values()}
            ticks = [int(v) for v in re.findall(r"\d+", repr(tick_clock.global_clock))]
            allocated = self.sems.allocated()
            waits = []
            for proc, handle in allocated.items():
                tick = ticks[proc] if proc < len(ticks) else 0
                if tick <= 0:
                    continue
                mult = 16 if handle.name.startswith("DMA") else 1
                waits.append((handle, tick * mult))
            for i in range(0, len(waits), 2):
                pair = waits[i : i + 2]
                ins = nc.sync.wait_ge(pair[0][0], pair[0][1])
                if len(pair) > 1:
                    ins._wait_ge(pair[1][0], pair[1][1])
            nc.sync.drain()
            # SP has now observed every engine's and DMA queue's final sem
            # value, so all work is done. A single SP->Pool handoff orders
            # the semaphore clear; the two stock all-engine barriers
            # (~2-3 us) are redundant.
            handoff = nc.alloc_semaphore("tail_handoff")
            nc.sync.sem_inc(handoff, 1)
            nc.gpsimd.wait_ge(handoff, 1)
            popped = nc._tile_sem_poison_stack.pop()
            assert popped is self._sem_poison
            nc.clear_and_free_semaphores(list(allocated.values()) + [handoff])

    return PatchedTileContext


def _split_multiwaits(nc, tc):
    """Move excess sem waits (>1 per instruction) onto same-engine
    EventSemaphore carriers inserted immediately before the consumer."""
    from concourse import mybir

    handles = tc.sem_handles
    eng_map = {
        mybir.EngineType.PE: nc.tensor,
        mybir.EngineType.DVE: nc.vector,
        mybir.EngineType.Activation: nc.scalar,
        mybir.EngineType.Pool: nc.gpsimd,
        mybir.EngineType.SP: nc.sync,
    }
    for f in nc.m.functions:
        for b in f.blocks:
            newlist = []
            changed = False
            for ins in list(b.instructions):
                si = ins.sync_info
                waits = list(si.on_wait) if si is not None and si.on_wait else []
                # EventSemaphore legally holds 2 waits (our own tail
                # carriers); don't touch it or rebuild its block.
                if type(ins).__name__ == "InstEventSemaphore":
                    newlist.append(ins)
                    continue
                if len(waits) > 1 and ins.engine in eng_map:
                    changed = True
                    extra, keep = waits[:-1], waits[-1:]
                    eng = eng_map[ins.engine]
                    for i in range(0, len(extra), 2):
                        pair = extra[i : i + 2]
                        carrier = eng.wait_ge(
                            handles[pair[0].ant_name], pair[0].wait_value
                        )
                        if len(pair) > 1:
                            carrier._wait_ge(
                                handles[pair[1].ant_name], pair[1].wait_value
                            )
                        # wait_ge appended the carrier to the current bb;
                        # pop it off there and splice it in before `ins`.
                        cb = nc.cur_bb.bb
                        cl = list(cb.instructions)
                        assert cl[-1].name == carrier.ins.name
                        cb.instructions = cl[:-1]
                        newlist.append(carrier.ins)
                    ins.sync_info = mybir.SyncInfo(on_wait=keep, on_update=si.on_update)
                newlist.append(ins)
            if changed:
                b.instructions = newlist


def _build():
    import concourse.bass as bass
    from concourse import mybir
    from concourse.masks import make_identity

    f32 = mybir.dt.float32
    f32r = mybir.dt.float32r

    nc = bass.Bass("TRN2", target_bir_lowering=False, debug=False, num_devices=N_CORES)
    inp = nc.dram_tensor("input_tensor", [1, U], f32, kind="ExternalInput").ap()
    state = nc.dram_tensor("state", [U, W], f32, kind="ExternalInput").ap()
    w = nc.dram_tensor("w", [2 * U + 1, U], f32, kind="ExternalInput").ap()
    out = nc.dram_tensor("out", [1, U], f32, kind="ExternalOutput").ap()

    PatchedTileContext = _patched_tile_context()
    with PatchedTileContext(nc) as tc:
        with (
            tc.tile_pool(name="data", bufs=1) as data,
            tc.tile_pool(name="work", bufs=2) as work,
            tc.tile_pool(name="psum", bufs=1, space="PSUM") as psum_pool,
        ):
            # --- loads: every DMA is a plain contiguous transfer. One
            # dma_start costs ~650ns of descriptor generation on its issuing
            # sequencer, so spread the gens across idle engine sequencers to
            # run them in parallel instead of serially on Sync.
            # One HWDGE/SWDGE stream moves only ~80-90 GB/s, so split the
            # 512KB state across four concurrent streams. Small dependency
            # DMAs are issued FIRST on their queue (queue FIFO order would
            # otherwise park them behind the big state transfers).
            # sync: inrow then state chunk 3; scalar: w_hi then wdot;
            # gpsimd/SWDGE (round-robin queues): state chunks 0-2.
            # State split into 6 concurrent ~85KB column streams, 2 per
            # issuing engine, interleaved so each compute half is fed by
            # each engine's FIRST state transfer. (Partition-split streams
            # with fat 4KB descriptors measured WORSE - a <128-partition DMA
            # only drives a fraction of the SBUF write ports.) Last NS
            # column (= input row transposed) comes from the PE q-column.
            # Issue order per engine: first-half state stream, then the
            # small dependency DMA, then the second-half stream - the first
            # stream's descriptor gen is the critical one, and the small
            # loads still complete well before their consumers need them.
            inrow = data.tile([1, U], f32, tag="inrow")
            nc.sync.dma_start(out=inrow[:], in_=inp[:], single_packet=True)
            wdot = data.tile([1, U], f32, tag="wdot")
            nc.scalar.dma_start(
                out=wdot[:], in_=w[2 * U : 2 * U + 1, :], single_packet=True
            )
            w_hi = data.tile([U, U], f32, tag="w_hi")
            nc.gpsimd.dma_start(out=w_hi[:], in_=w[0:U, :])

            ns = data.tile([U, W], f32, tag="ns")
            six = [0, 171, 342, 512, 683, 853, 1023]
            stream_eng = [nc.gpsimd, nc.sync, nc.scalar]
            for k in range(6):
                lo, hi = six[k], six[k + 1]
                eng = stream_eng[k % 3]
                eng.dma_start(out=ns[:, lo:hi], in_=state[:, lo + 1 : hi + 1])

            # constants on otherwise-idle engines
            one = data.tile([1, 1], f32, tag="one")
            nc.vector.memset(one[:], 1.0)
            ident = data.tile([U, U], f32, tag="ident")
            make_identity(nc, ident[:])

            # Preload the ACT exp table at kernel start (otherwise the
            # ~1.5us table load serializes in front of the first real exp).
            act_warm = data.tile([1, 1], f32, tag="act_warm")
            nc.scalar.activation(
                act_warm[:], one[:], mybir.ActivationFunctionType.Exp
            )

            # q column [U,1] = input row transposed, via K=1 matmul with 1.0
            q_psum = psum_pool.tile([U, 1], f32, tag="q_psum")
            nc.tensor.matmul(q_psum[:], lhsT=inrow[:], rhs=one[:], start=True, stop=True)
            nc.vector.tensor_copy(ns[:, W - 1 : W], q_psum[:])

            # M_eff = w_hi + outer(q, w_dot); outer via K=1 matmul on PE
            outer_psum = psum_pool.tile([U, U], f32, tag="outer")
            nc.tensor.matmul(
                outer_psum[:], lhsT=inrow[:], rhs=wdot[:], start=True, stop=True
            )
            meff = data.tile([U, U], f32r, tag="meff")
            nc.vector.tensor_add(meff[:], w_hi[:], outer_psum[:])

            # Keep the PE clock ramped while waiting for the state DMA: a few
            # dummy K=1 matmuls into a scratch bank (PE would otherwise idle
            # ~1us and drop back to the slow p-state for the big matmuls).
            warm_psum = psum_pool.tile([U, U], f32, tag="warm")
            for _ in range(2):
                nc.tensor.matmul(
                    warm_psum[:], lhsT=inrow[:], rhs=wdot[:], start=True, stop=True
                )

            # f32r copy of NS for the PE (f32r needs pre-rounded input), on
            # DVE (Pool's CAST path measured ~3.7 ns/elem; DVE is ~1 ns/elem).
            nsr = data.tile([U, W], f32r, tag="nsr")
            warmed = False

            l_all = data.tile([U, N_CHUNK], f32, tag="l_all")
            num_all = data.tile([U, N_CHUNK], f32, tag="num_all")

            for c in range(N_CHUNK):
                lo, hi = c * CHUNK, (c + 1) * CHUNK
                nc.vector.tensor_copy(nsr[:, lo:hi], ns[:, lo:hi])
                ps = psum_pool.tile([U, CHUNK], f32, tag=f"ps{c}")
                nc.tensor.matmul(
                    ps[:], lhsT=meff[:], rhs=nsr[:, lo:hi], start=True, stop=True
                )
                e = work.tile([U, CHUNK], f32, tag="e")
                nc.scalar.activation(
                    e[:],
                    ps[:],
                    mybir.ActivationFunctionType.Exp,
                    accum_out=l_all[:, c : c + 1],
                )
                # num_c[u] = sum_j NS[u,j]*E[u,j]: out=(NS*1.0)*E, accum=sum
                t = work.tile([U, CHUNK], f32, tag="t")
                nc.vector.scalar_tensor_tensor(
                    out=t[:],
                    in0=ns[:, lo:hi],
                    scalar=1.0,
                    in1=e[:],
                    op0=mybir.AluOpType.mult,
                    op1=mybir.AluOpType.mult,
                    accum_out=num_all[:, c : c + 1],
                )

            l_sum = data.tile([U, 1], f32, tag="l_sum")
            nc.vector.reduce_sum(l_sum[:], l_all[:], axis=mybir.AxisListType.X)
            num_sum = data.tile([U, 1], f32, tag="num_sum")
            nc.vector.reduce_sum(num_sum[:], num_all[:], axis=mybir.AxisListType.X)
            r = data.tile([U, 1], f32, tag="r")
            nc.vector.reciprocal(r[:], l_sum[:])
            cfin = data.tile([U, 1], f32, tag="cfin")
            nc.vector.tensor_mul(cfin[:], num_sum[:], r[:])

            # transpose c to a contiguous [1,U] row on the PE, then one
            # contiguous 512B DMA out, straight from PSUM
            crow_psum = psum_pool.tile([1, U], f32, tag="crow")
            nc.tensor.matmul(
                crow_psum[:], lhsT=cfin[:], rhs=ident[:], start=True, stop=True
            )
            crow = data.tile([1, U], f32, tag="crow_sb")
            nc.scalar.copy(crow[:], crow_psum[:])
            nc.sync.dma_start(out=out[:], in_=crow[:], single_packet=True)

    _split_multiwaits(nc, tc)
    return nc


def _get_nc():
    if "nc" not in _cache:
        _cache["nc"] = _build()
    return _cache["nc"]


def kernel(**inputs) -> np.ndarray:
    from concourse.bass_utils import run_bass_kernel_spmd

    nc = _get_nc()
    in_map = {
        "input_tensor": np.ascontiguousarray(inputs["input_tensor"], dtype=np.float32),
        "state": np.ascontiguousarray(inputs["state"], dtype=np.float32),
        "w": np.ascontiguousarray(inputs["w"], dtype=np.float32),
    }
    in_maps = [in_map for _ in range(N_CORES)]
    res = run_bass_kernel_spmd(nc, in_maps, list(range(N_CORES)))
    return np.asarray(res.results[0]["out"], dtype=np.float32)


# revision 53
# speedup vs baseline: 1.1080x; 1.0026x over previous
"""Trainium2 Bass kernel for nn_RSA_layer (RSA relational self-attention layer).

The reference builds a [W, W, U] sim/softmax tensor but returns only row
i = W-1 of the weighted sum c. Two algebraic reductions make the kernel
tiny:

1. Only query row i = W-1 matters, and the softmax terms constant over the
   key axis j (proj_hj[i, u] and b[u]) cancel in the softmax ratio, so
       s[j, u] = (fs @ w_hi)[j, u] + (fs[W-1] . fs[j]) * w_dot[u]
2. The rank-1 dot-product term folds into the matmul weights:
       s = ((w_hi + outer(q, w_dot)).T @ NS)           with q = fs[W-1]
   where NS = new_state = [state[:, 1:] | input^T]  ([U, W], u on
   partitions, j on the free axis).

Then per unit u (one SBUF partition), softmax over j is a row softmax:
   c[u] = sum_j NS[u,j] e^{s[u,j]} / sum_j e^{s[u,j]}
computed without max subtraction (|s| <= ~30, safely inside f32 range).

Implementation notes (HW-profile driven):
- All DMAs are contiguous HWDGE transfers. Partition-scatter/gather DMAs
  ([1,128] <-> [128,1]) cost ~6-7 us (128 x 4B descriptors) and are
  replaced by tiny PE matmuls: q column via matmul with a [1,1] ones
  tile, output row via matmul with a [128,128] identity.
- The score matmul runs in float32r (1 cycle/row vs fp32's 4). f32r
  operands must be "rounded": the Pool engine copy-casts the f32 NS
  tiles to f32r off the critical path.
- exp runs on ACT reading PSUM directly with accum_out producing the
  softmax denominator for free; the numerator is a fused
  scalar_tensor_tensor (out=(NS*1.0)*E, accum_out=sum) on DVE.

SPMD strategy: the problem is ~650 KB of traffic and ~35 MFLOP - far
below the ~7-20 us on-chip collective latency floor - so each of the 8
cores computes the full (reduced) answer independently and core 0's
output is returned. No cross-core communication.
"""

import re

import numpy as np

W = 1024
U = 128
N_CORES = 8
CHUNK = 512
N_CHUNK = W // CHUNK

_cache: dict = {}


def _patched_tile_context():
    import concourse.tile as tile

    class PatchedTileContext(tile.TileContext):
        """TileContext whose tail drain splits its semaphore waits.

        The walrus build here encodes at most 1 sem wait per regular
        instruction (2 per EventSemaphore), but stock Tile attaches every
        outstanding sem wait to one tail Drain. Emit standalone 2-wait
        EventSemaphore SP instructions covering the final vector clock
        instead, then a bare drain. Also stashes live sem handles for the
        post-exit multi-wait splitting pass (_split_multiwaits).
        """

        def _drain_and_barrier(self, tick_clock, wait_clock):
            nc = self.nc
            self.sem_handles = {h.name: h for h in self.sems.allocated().values()}
            ticks = [int(v) for v in re.findall(r"\d+", repr(tick_clock.global_clock))]
            allocated = self.sems.allocated()
            waits = []
            for proc, handle in allocated.items():
                tick = ticks[proc] if proc < len(ticks) else 0
                if tick <= 0:
                    continue
                mult = 16 if handle.name.startswith("DMA") else 1
                waits.append((handle, tick * mult))
            for i in range(0, len(waits), 2):
                pair = waits[i : i + 2]
                ins = nc.sync.wait_ge(pair[0][0], pair[0][1])
                if len(pair) > 1:
                    ins._wait_ge(pair[1][0], pair[1][1])
            nc.sync.drain()
            # SP has now observed every engine's and DMA queue's final sem
            # value, so all work is done. A single SP->Pool handoff orders
            # the semaphore clear; the two stock all-engine barriers
            # (~2-3 us) are redundant.
            handoff = nc.alloc_semaphore("tail_handoff")
            nc.sync.sem_inc(handoff, 1)
            nc.gpsimd.wait_ge(handoff, 1)
            popped = nc._tile_sem_poison_stack.pop()
            assert popped is self._sem_poison
            nc.clear_and_free_semaphores(list(allocated.values()) + [handoff])

    return PatchedTileContext


def _split_multiwaits(nc, tc):
    """Move excess sem waits (>1 per instruction) onto same-engine
    EventSemaphore carriers inserted immediately before the consumer."""
    from concourse import mybir

    handles = tc.sem_handles
    eng_map = {
        mybir.EngineType.PE: nc.tensor,
        mybir.EngineType.DVE: nc.vector,
        mybir.EngineType.Activation: nc.scalar,
        mybir.EngineType.Pool: nc.gpsimd,
        mybir.EngineType.SP: nc.sync,
    }
    for f in nc.m.functions:
        for b in f.blocks:
            newlist = []
            changed = False
            for ins in list(b.instructions):
                si = ins.sync_info
                waits = list(si.on_wait) if si is not None and si.on_wait else []
                # EventSemaphore legally holds 2 waits (our own tail
                # carriers); don't touch it or rebuild its block.
                if type(ins).__name__ == "InstEventSemaphore":
                    newlist.append(ins)
                    continue
                if len(waits) > 1 and ins.engine in eng_map:
                    changed = True
                    extra, keep = waits[:-1], waits[-1:]
                    eng = eng_map[ins.engine]
                    for i in range(0, len(extra), 2):
                        pair = extra[i : i + 2]
                        carrier = eng.wait_ge(
                            handles[pair[0].ant_name], pair[0].wait_value
                        )
                        if len(pair) > 1:
                            carrier._wait_ge(
                                handles[pair[1].ant_name], pair[1].wait_value
                            )
                        # wait_ge appended the carrier to the current bb;
                        # pop it off there and splice it in before `ins`.
                        cb = nc.cur_bb.bb
                        cl = list(cb.instructions)
                        assert cl[-1].name == carrier.ins.name
                        cb.instructions = cl[:-1]
                        newlist.append(carrier.ins)
                    ins.sync_info = mybir.SyncInfo(on_wait=keep, on_update=si.on_update)
                newlist.append(ins)
            if changed:
                b.instructions = newlist


def _build():
    import concourse.bass as bass
    from concourse import mybir
    f32 = mybir.dt.float32
    f32r = mybir.dt.float32r

    nc = bass.Bass("TRN2", target_bir_lowering=False, debug=False, num_devices=N_CORES)
    inp = nc.dram_tensor("input_tensor", [1, U], f32, kind="ExternalInput").ap()
    state = nc.dram_tensor("state", [U, W], f32, kind="ExternalInput").ap()
    w = nc.dram_tensor("w", [2 * U + 1, U], f32, kind="ExternalInput").ap()
    out = nc.dram_tensor("out", [1, U], f32, kind="ExternalOutput").ap()

    PatchedTileContext = _patched_tile_context()
    with PatchedTileContext(nc) as tc:
        with (
            tc.tile_pool(name="data", bufs=1) as data,
            tc.tile_pool(name="work", bufs=2) as work,
            tc.tile_pool(name="psum", bufs=1, space="PSUM") as psum_pool,
        ):
            # --- loads: every DMA is a plain contiguous transfer. One
            # dma_start costs ~650ns of descriptor generation on its issuing
            # sequencer, so spread the gens across idle engine sequencers to
            # run them in parallel instead of serially on Sync.
            # One HWDGE/SWDGE stream moves only ~80-90 GB/s, so split the
            # 512KB state across four concurrent streams. Small dependency
            # DMAs are issued FIRST on their queue (queue FIFO order would
            # otherwise park them behind the big state transfers).
            # sync: inrow then state chunk 3; scalar: w_hi then wdot;
            # gpsimd/SWDGE (round-robin queues): state chunks 0-2.
            # State split into 6 concurrent ~85KB column streams, 2 per
            # issuing engine, interleaved so each compute half is fed by
            # each engine's FIRST state transfer. (Partition-split streams
            # with fat 4KB descriptors measured WORSE - a <128-partition DMA
            # only drives a fraction of the SBUF write ports.) Last NS
            # column (= input row transposed) comes from the PE q-column.
            # Issue order per engine: first-half state stream, then the
            # small dependency DMA, then the second-half stream - the first
            # stream's descriptor gen is the critical one, and the small
            # loads still complete well before their consumers need them.
            # input row scattered as 4x32 pieces onto partitions 0/32/64/96
            # (4 fat descriptors); a DVE 32x32 block-transpose then yields
            # the q column. A direct [1,128]->[128,1] DMA would be 128 4-byte
            # descriptors (~6-7us).
            qt = data.tile([U, 32], f32, tag="qt")
            # partition step is in elements: 32 partitions x 32-elem rows
            row_pitch = qt[:].ap[0][0]
            qt_quads = bass.AP(
                tensor=qt.tensor, offset=qt.offset, ap=[[32 * row_pitch, 4], [1, 32]]
            )
            inp_quads = bass.AP(
                tensor=inp.tensor, offset=inp.offset, ap=[[32, 4], [1, 32]]
            )
            nc.sync.dma_start(out=qt_quads, in_=inp_quads)
            # w_dot row broadcast to all 128 partitions (stride-0 DRAM read)
            wdot_row = w[2 * U]
            wdb = data.tile([U, U], f32, tag="wdb")
            nc.scalar.dma_start(
                out=wdb[:],
                in_=bass.AP(
                    tensor=wdot_row.tensor,
                    offset=wdot_row.offset,
                    ap=[[0, U]] + list(wdot_row.ap),
                ),
            )
            w_hi = data.tile([U, U], f32, tag="w_hi")
            nc.gpsimd.dma_start(out=w_hi[:], in_=w[0:U, :])

            ns = data.tile([U, W], f32, tag="ns")
            six = [0, 171, 342, 512, 683, 853, 1023]
            stream_eng = [nc.gpsimd, nc.sync, nc.scalar]
            for k in range(6):
                lo, hi = six[k], six[k + 1]
                eng = stream_eng[k % 3]
                eng.dma_start(out=ns[:, lo:hi], in_=state[:, lo + 1 : hi + 1])

            # Preload the ACT exp table at kernel start (otherwise the
            # ~1.5us table load serializes in front of the first real exp).
            act_warm = data.tile([1, 1], f32, tag="act_warm")
            nc.scalar.activation(
                act_warm[:], qt[0:1, 0:1], mybir.ActivationFunctionType.Exp
            )

            # q column via DVE block transpose; copy into the last NS column
            qtt = data.tile([U, 32], f32, tag="qtt")
            nc.vector.transpose(qtt[:], qt[:])
            nc.vector.tensor_copy(ns[:, W - 1 : W], qtt[:, 0:1])

            # M_eff[u,u'] = w_hi[u,u'] + q[u] * w_dot[u'], one fused DVE op
            meff = data.tile([U, U], f32r, tag="meff")
            nc.vector.scalar_tensor_tensor(
                out=meff[:],
                in0=wdb[:],
                scalar=qtt[:, 0:1],
                in1=w_hi[:],
                op0=mybir.AluOpType.mult,
                op1=mybir.AluOpType.add,
            )

            # Keep the PE clock ramped while waiting for the state DMA: a few
            # dummy matmuls into a scratch bank (PE would otherwise start the
            # big matmuls at the cold p-state).
            warm_psum = psum_pool.tile([1, U], f32, tag="warm")
            for _ in range(2):
                nc.tensor.matmul(
                    warm_psum[:],
                    lhsT=w_hi[:, 0:1],
                    rhs=w_hi[:, 0:U],
                    start=True,
                    stop=True,
                )

            # f32r copy of NS for the PE (f32r needs pre-rounded input), on
            # DVE (Pool's CAST path measured ~3.7 ns/elem; DVE is ~1 ns/elem).
            nsr = data.tile([U, W], f32r, tag="nsr")
            warmed = False

            l_all = data.tile([U, N_CHUNK], f32, tag="l_all")
            num_all = data.tile([U, N_CHUNK], f32, tag="num_all")

            for c in range(N_CHUNK):
                lo, hi = c * CHUNK, (c + 1) * CHUNK
                nc.vector.tensor_copy(nsr[:, lo:hi], ns[:, lo:hi])
                ps = psum_pool.tile([U, CHUNK], f32, tag=f"ps{c}")
                nc.tensor.matmul(
                    ps[:], lhsT=meff[:], rhs=nsr[:, lo:hi], start=True, stop=True
                )
                e = work.tile([U, CHUNK], f32, tag="e")
                nc.scalar.activation(
                    e[:],
                    ps[:],
                    mybir.ActivationFunctionType.Exp,
                    accum_out=l_all[:, c : c + 1],
                )
                # num_c[u] = sum_j NS[u,j]*E[u,j]: out=(NS*1.0)*E, accum=sum
                t = work.tile([U, CHUNK], f32, tag="t")
                nc.vector.scalar_tensor_tensor(
                    out=t[:],
                    in0=ns[:, lo:hi],
                    scalar=1.0,
                    in1=e[:],
                    op0=mybir.AluOpType.mult,
                    op1=mybir.AluOpType.mult,
                    accum_out=num_all[:, c : c + 1],
                )

            l_sum = data.tile([U, 1], f32, tag="l_sum")
            nc.vector.reduce_sum(l_sum[:], l_all[:], axis=mybir.AxisListType.X)
            num_sum = data.tile([U, 1], f32, tag="num_sum")
            nc.vector.reduce_sum(num_sum[:], num_all[:], axis=mybir.AxisListType.X)
            # c = num * (1/l) straight into column 0 of a transpose scratch
            r = data.tile([U, 1], f32, tag="r")
            nc.vector.reciprocal(r[:], l_sum[:])
            cf = data.tile([U, 32], f32, tag="cf")
            nc.vector.tensor_mul(cf[:, 0:1], num_sum[:], r[:])
            # DVE block transpose lands c as 4x32 row pieces on partitions
            # 0/32/64/96; one 4-descriptor DMA writes the contiguous row out
            cfr = data.tile([U, 32], f32, tag="cfr")
            nc.vector.transpose(cfr[:], cf[:])
            cfr_quads = bass.AP(
                tensor=cfr.tensor,
                offset=cfr.offset,
                ap=[[32 * cfr[:].ap[0][0], 4], [1, 32]],
            )
            out_quads = bass.AP(
                tensor=out.tensor, offset=out.offset, ap=[[32, 4], [1, 32]]
            )
            nc.sync.dma_start(out=out_quads, in_=cfr_quads)

    _split_multiwaits(nc, tc)
    return nc


def _get_nc():
    if "nc" not in _cache:
        _cache["nc"] = _build()
    return _cache["nc"]


def kernel(**inputs) -> np.ndarray:
    from concourse.bass_utils import run_bass_kernel_spmd

    nc = _get_nc()
    in_map = {
        "input_tensor": np.ascontiguousarray(inputs["input_tensor"], dtype=np.float32),
        "state": np.ascontiguousarray(inputs["state"], dtype=np.float32),
        "w": np.ascontiguousarray(inputs["w"], dtype=np.float32),
    }
    in_maps = [in_map for _ in range(N_CORES)]
    res = run_bass_kernel_spmd(nc, in_maps, list(range(N_CORES)))
    return np.asarray(res.results[0]["out"], dtype=np.float32)
